# revision 11
# baseline (speedup 1.0000x reference)
"""Trainium2 Bass kernel for the memristor-crossbar layer (nn_CustomLayer_30588757082254).

out = unmap(x @ G_eff) + bias, where G_eff = 1/(1/G + R_par) is an elementwise
transform of weight.T with globally min/max-normalized conductances.

Strategy: data-parallel over batch (8 cores x 1024 rows). Single-phase,
column-chunked pipeline: the N=2048 output columns are processed in 4 chunks
of 512; chunk ch+1's conductance transform runs on ACT/DVE/Pool while chunk
ch's matmuls run on the PE with full-K accumulation in PSUM (all 8 banks, one
per m-block), so there is no DRAM staging of partial products and x is loaded
exactly once.

Math (S = 1/s folds the unmapping scale; kappa = wmin - g_min*S = -a):
  s = (g_max-g_min)/(wmax-wmin);  a = g_min/s - wmin
  geff'  = 1/( 1/(WT+a) + s*R )        (R = 4098 + 2n - 2p - 256*kt)
  geff'' = geff' + kappa               (folds the xs*kappa row-sum term:
                                        x @ (kappa*ones) = xs*kappa)
  out    = x @ geff'' + bias

Folding kappa into the operand eliminates both the row-sum (xs) matmuls and
the rank-1 bias matmuls from the PE; bias is added in a DVE tensor-tensor
epilogue straight out of PSUM. The +a shift rides on the host's wt relayout
(wt input = weight.T + a). s*R is split into a per-(partition,ktile) part
(ACT bias table, [128,16]) and a pure column part 2*s*n (Pool tensor-tensor,
the only elementwise op GPSIMD supports - no PSUM, no TensorScalarPtr).
Transform per k-tile, in-place on the streamed wt tile:
  DVE recip -> ACT +row -> Pool +col -> DVE recip -> ACT +kappa (f32r out).
Each engine carries <= ~26us per 27.3us matmul chunk, so the PE stays the
bottleneck. Emission is stage-major per chunk (all recips, then all row-adds,
...) so the in-order engine queues pipeline across k-tiles; the wt ring must
therefore hold a full chunk (18 bufs > 16 k-tiles) or the queues deadlock.
"""
import numpy as np

import concourse.bass as bass
import concourse.mybir as mybir
import concourse.tile as tile
from concourse import bacc
from concourse.bass_utils import run_bass_kernel_spmd
from concourse.dve_ops import RECIP_APPROX_FAST_CONSTS, RECIPROCAL_APPROX_FAST

F32 = mybir.dt.float32
F32R = mybir.dt.float32r
AF = mybir.ActivationFunctionType
ALU = mybir.AluOpType
CRC = RECIP_APPROX_FAST_CONSTS

N_CORES = 8
B, K, N = 8192, 2048, 2048
BC = B // N_CORES            # 1024 batch rows per core
KT = K // 128                # 16 k-tiles
MB = BC // 128               # 8 m-blocks per core
W = 512                      # output column chunk width
NCH = N // W                 # 4 chunks
XG = 4                       # x loaded in 4 groups of 4 k-tiles

PARASITIC_R = 2.0
G_MIN, G_MAX = 1.0 / 100000.0, 1.0 / 1000.0

_CACHE = {}


def _build_nc():
    nc = bacc.Bacc("TRN2", target_bir_lowering=False, debug=False,
                   num_devices=N_CORES)
    wt_in = nc.dram_tensor("wt", [K, N], F32, kind="ExternalInput")
    xt_in = nc.dram_tensor("xt", [K, BC], F32R, kind="ExternalInput")
    bias_in = nc.dram_tensor("bias", [128, N], F32, kind="ExternalInput")
    mmx_in = nc.dram_tensor("mmx", [128, 20], F32, kind="ExternalInput")
    cv_in = nc.dram_tensor("cv", [128, N], F32, kind="ExternalInput")
    out_d = nc.dram_tensor("out", [BC, N], F32, kind="ExternalOutput")

    # x group g holds k rows [g*512, (g+1)*512) as [128p, 4kt, 1024m]
    xt_r = xt_in.rearrange("(g j p) m -> g p j m", g=XG, p=128)

    with tile.TileContext(nc) as tc:
        with (
            tc.tile_pool(name="xkp", bufs=1) as xkp,
            tc.tile_pool(name="cvp", bufs=1) as cvp,
            tc.tile_pool(name="biasp", bufs=1) as biasp,
            tc.tile_pool(name="smallp", bufs=1) as sp,
            tc.tile_pool(name="wtp", bufs=26) as wtp,
            tc.tile_pool(name="geffp", bufs=2) as geffp,
            tc.tile_pool(name="osbp", bufs=4) as osbp,
            tc.tile_pool(name="pcp", bufs=1, space="PSUM") as pcp,
        ):
            # ---------------- tiny inputs (host pre-broadcast) ----------------
            with nc.named_scope("setup"):
                bcv = sp.tile([128, 20], F32, tag="bcv")
                nc.sync.dma_start(out=bcv[:], in_=mmx_in[:])
                cv2s = cvp.tile([128, N], F32, tag="cv2s")
                nc.gpsimd.dma_start(out=cv2s[:], in_=cv_in[:])
                bias_sb = biasp.tile([128, N], F32, tag="bias_sb")
                # warm the engines during the preamble: first custom-DVE op
                # triggers a library load (~5us) and first activation a table
                # load (~1.3us); pay both on a dummy tile before wt arrives.
                wu = sp.tile([128, 8], F32, tag="wu")
                nc.vector.memset(wu[:], 1.0)
                nc.vector._custom_dve(RECIPROCAL_APPROX_FAST, out=wu[:],
                                      in0=wu[:], s0=CRC["s0"], s1=CRC["s1"],
                                      imm2=CRC["imm2"])
                nc.scalar.activation(wu[:], wu[:], AF.Identity, bias=0.0,
                                     scale=1.0)
                nc.gpsimd.tensor_tensor(wu[:], wu[:], wu[:], ALU.add)
            kap_b = bcv[:, 2:3]
            rowb = bcv[:, 4:20]          # [128, kt]: s*(4098 - 256*kt - 2p)

            xg = {}
            geff = {}

            wtiles = {}

            ATA_KTS = {1: set(range(10, 16)), 2: {14, 15}}

            def tf_front(ch, kts, interleave_x=False):
                """dma + recip1 + row-add for the given k-tiles of chunk ch.

                k-tiles in ATA_KTS[ch] skip the ACT row-add here; tf_back
                applies row+col in one fused DVE affine_then_add instead,
                trimming the Pool-serial chain on the critical chunks."""
                for kt in kts:
                    w_t = wtp.tile([128, W], F32, tag="wt",
                                   name=f"wt{ch}_{kt}")
                    nc.sync.dma_start(
                        out=w_t[:],
                        in_=wt_in[kt * 128:(kt + 1) * 128,
                                  ch * W:(ch + 1) * W])
                    if interleave_x and kt % 4 == 3:
                        g = kt // 4
                        xg[g] = xkp.tile([128, 4, BC], F32R, tag=f"xg{g}",
                                         name=f"xg{g}")
                        nc.sync.dma_start(out=xg[g][:], in_=xt_r[g])
                    wtiles[ch, kt] = w_t
                for kt in kts:
                    nc.vector._custom_dve(RECIPROCAL_APPROX_FAST,
                                          out=wtiles[ch, kt][:],
                                          in0=wtiles[ch, kt][:],
                                          s0=CRC["s0"], s1=CRC["s1"],
                                          imm2=CRC["imm2"])
                for kt in kts:
                    if kt in ATA_KTS.get(ch, ()):
                        continue
                    nc.scalar.activation(wtiles[ch, kt][:], wtiles[ch, kt][:],
                                         AF.Identity,
                                         bias=rowb[:, kt:kt + 1], scale=1.0)

            def tf_back(ch):
                """col-add + recip2 + kappa for all k-tiles of chunk ch."""
                ata = ATA_KTS.get(ch, set())
                for kt in sorted(ata):
                    nc.vector.affine_then_add(wtiles[ch, kt][:],
                                              wtiles[ch, kt][:],
                                              cv2s[:, ch * W:(ch + 1) * W],
                                              1.0, rowb[:, kt:kt + 1])
                for kt in range(KT):
                    if kt in ata:
                        continue
                    nc.gpsimd.tensor_tensor(wtiles[ch, kt][:],
                                            wtiles[ch, kt][:],
                                            cv2s[:, ch * W:(ch + 1) * W],
                                            ALU.add)
                for kt in range(KT):
                    nc.vector._custom_dve(RECIPROCAL_APPROX_FAST,
                                          out=wtiles[ch, kt][:],
                                          in0=wtiles[ch, kt][:],
                                          s0=CRC["s0"], s1=CRC["s1"],
                                          imm2=CRC["imm2"])
                for kt in range(KT):
                    ge_t = geffp.tile([128, W], F32R, tag=f"ge{kt}",
                                      name=f"ge{ch}_{kt}")
                    nc.scalar.activation(ge_t[:], wtiles[ch, kt][:],
                                         AF.Identity, bias=kap_b, scale=1.0)
                    geff[ch, kt] = ge_t

            def emit_epilogue(ch, mb, pc):
                # GPSIMD cannot read PSUM, so the bias add rides on DVE
                osb = osbp.tile([128, W], F32, tag="osb", name=f"ep{ch}_{mb}")
                nc.vector.tensor_tensor(osb[:], pc[:],
                                        bias_sb[:, ch * W:(ch + 1) * W],
                                        ALU.add)
                nc.sync.dma_start(
                    out=out_d[mb * 128:(mb + 1) * 128,
                              ch * W:(ch + 1) * W],
                    in_=osb[:])

            def one_mm(pc, ch, kt, mb):
                g, j = divmod(kt, 4)
                nc.tensor.matmul(
                    pc[:], xg[g][:, j, mb * 128:(mb + 1) * 128],
                    geff[ch, kt][:], start=(kt == 0), stop=(kt == KT - 1))

            def emit_mm(ch, kt_outer):
                pcs = {mb: pcp.tile([128, W], F32, tag=f"pc{mb}",
                                    name=f"pc{ch}_{mb}") for mb in range(MB)}
                if kt_outer:
                    # chunk 0: x and geff stream in k-order; consume them as
                    # they land so the PE never waits on the chunk tail.
                    for kt in range(KT):
                        for mb in range(MB):
                            one_mm(pcs[mb], ch, kt, mb)
                    for mb in range(MB):
                        emit_epilogue(ch, mb, pcs[mb])
                else:
                    # steady state: mb-outer staggers PSUM-bank drains so the
                    # epilogue and out-DMA spread across the chunk.
                    for mb in range(MB):
                        for kt in range(KT):
                            one_mm(pcs[mb], ch, kt, mb)
                        emit_epilogue(ch, mb, pcs[mb])

            HALF = KT // 2
            with nc.named_scope("tf0"):
                tf_front(0, range(KT), interleave_x=True)
            with nc.named_scope("tf1a"):
                tf_front(1, range(HALF))
            with nc.named_scope("tf0b"):
                tf_back(0)
            # bias lands well after the chunk-0 DMA crunch, before epilogue 0
            nc.sync.dma_start(out=bias_sb[:], in_=bias_in[:])
            # transform pipeline runs one full chunk ahead of the mm stream so
            # mm(ch)'s PSUM-draining epilogues never block tf(ch+1) recips
            with nc.named_scope("tf1b"):
                tf_front(1, range(HALF, KT))
                tf_front(2, range(HALF))
                tf_back(1)
            for ch in range(NCH):
                with nc.named_scope(f"mm{ch}"):
                    emit_mm(ch, kt_outer=(ch == 0))
                if ch + 2 < NCH:
                    with nc.named_scope(f"tf{ch + 2}b"):
                        tf_front(ch + 2, range(HALF, KT))
                        if ch + 3 < NCH:
                            tf_front(ch + 3, range(HALF))
                        tf_back(ch + 2)
    nc.finalize()
    return nc


def _prep_inputs(x, weight, bias):
    wmin = float(weight.min())
    wmax = float(weight.max())
    s = (G_MAX - G_MIN) / (wmax - wmin)
    a = G_MIN / s - wmin
    kappa = wmin - G_MIN / s
    # the +a shift rides on the wt relayout (transposed, contiguous, shifted)
    wtT = np.ascontiguousarray(weight.T + np.float32(a))
    mmx = np.zeros((128, 20), dtype=np.float32)
    mmx[:, 0] = s
    mmx[:, 1] = a
    mmx[:, 2] = kappa
    # per-(partition, ktile) parasitic term: s*(4098 - 256*kt - 2p)
    p = np.arange(128, dtype=np.float64)[:, None]
    kt = np.arange(KT, dtype=np.float64)[None, :]
    mmx[:, 4:20] = (s * (4098.0 - 256.0 * kt - 2.0 * p)).astype(np.float32)
    mmx = np.ascontiguousarray(mmx)
    # pure column part of the parasitic term: cv[p, n] = 2*s*n
    cv = np.ascontiguousarray(np.broadcast_to(
        (np.float64(s) * 2.0 * np.arange(N, dtype=np.float64))[None, :]
        .astype(np.float32), (128, N)))

    biasb = np.ascontiguousarray(
        np.broadcast_to(bias.reshape(1, N), (128, N))).astype(np.float32)
    in_maps = []
    for c in range(N_CORES):
        x_c = x[c * BC:(c + 1) * BC, :]           # [BC, K]
        xt_c = np.ascontiguousarray(x_c.T)
        in_maps.append({"wt": wtT, "xt": xt_c, "bias": biasb, "mmx": mmx,
                        "cv": cv})
    return in_maps


def _run(x, weight, bias, trace=False, trace_kwargs=None):
    if "nc" not in _CACHE:
        _CACHE["nc"] = _build_nc()
    nc = _CACHE["nc"]
    in_maps = _prep_inputs(x, weight, bias)
    res = run_bass_kernel_spmd(nc, in_maps, list(range(N_CORES)), trace=trace,
                               **(trace_kwargs or {}))
    out = np.concatenate([res.results[c]["out"] for c in range(N_CORES)], axis=0)
    return out, res


def kernel(x, weight, bias):
    x = np.asarray(x, dtype=np.float32)
    weight = np.asarray(weight, dtype=np.float32)
    bias = np.asarray(bias, dtype=np.float32)
    out, _ = _run(x, weight, bias, trace=False)
    return out.astype(np.float32)


# revision 15
# speedup vs baseline: 1.0359x; 1.0359x over previous
"""Trainium2 Bass kernel for the memristor-crossbar layer (nn_CustomLayer_30588757082254).

out = unmap(x @ G_eff) + bias, where G_eff = 1/(1/G + R_par) is an elementwise
transform of weight.T with globally min/max-normalized conductances.

Strategy: data-parallel over batch (8 cores x 1024 rows). Single-phase,
column-chunked pipeline: the N=2048 output columns are processed in 4 chunks
of 512; chunk ch+1's conductance transform runs on ACT/DVE/Pool while chunk
ch's matmuls run on the PE with full-K accumulation in PSUM (all 8 banks, one
per m-block), so there is no DRAM staging of partial products and x is loaded
exactly once.

Math (S = 1/s folds the unmapping scale; kappa = wmin - g_min*S = -a):
  s = (g_max-g_min)/(wmax-wmin);  a = g_min/s - wmin
  geff'  = 1/( 1/(WT+a) + s*R )        (R = 4098 + 2n - 2p - 256*kt)
  geff'' = geff' + kappa               (folds the xs*kappa row-sum term:
                                        x @ (kappa*ones) = xs*kappa)
  out    = x @ geff'' + bias

Folding kappa into the operand eliminates both the row-sum (xs) matmuls and
the rank-1 bias matmuls from the PE; bias is added in a DVE tensor-tensor
epilogue straight out of PSUM. The +a shift rides on the host's wt relayout
(wt input = weight.T + a). s*R is split into a per-(partition,ktile) part
(ACT bias table, [128,16]) and a pure column part 2*s*n (Pool tensor-tensor,
the only elementwise op GPSIMD supports - no PSUM, no TensorScalarPtr).
Transform per k-tile, in-place on the streamed wt tile:
  DVE recip -> ACT +row -> Pool +col -> DVE recip -> ACT +kappa (f32r out).
Each engine carries <= ~26us per 27.3us matmul chunk, so the PE stays the
bottleneck. Emission is stage-major per chunk (all recips, then all row-adds,
...) so the in-order engine queues pipeline across k-tiles; the wt ring must
therefore hold a full chunk (18 bufs > 16 k-tiles) or the queues deadlock.
"""
import numpy as np

import concourse.bass as bass
import concourse.mybir as mybir
import concourse.tile as tile
from concourse import bacc
from concourse.bass_utils import run_bass_kernel_spmd
from concourse.dve_ops import RECIP_APPROX_FAST_CONSTS, RECIPROCAL_APPROX_FAST

F32 = mybir.dt.float32
F32R = mybir.dt.float32r
AF = mybir.ActivationFunctionType
ALU = mybir.AluOpType
CRC = RECIP_APPROX_FAST_CONSTS

N_CORES = 8
B, K, N = 8192, 2048, 2048
BC = B // N_CORES            # 1024 batch rows per core
KT = K // 128                # 16 k-tiles
MB = BC // 128               # 8 m-blocks per core
W = 512                      # output column chunk width
NCH = N // W                 # 4 chunks
XG = 4                       # x loaded in 4 groups of 4 k-tiles

PARASITIC_R = 2.0
G_MIN, G_MAX = 1.0 / 100000.0, 1.0 / 1000.0

_CACHE = {}


def _build_nc():
    nc = bacc.Bacc("TRN2", target_bir_lowering=False, debug=False,
                   num_devices=N_CORES)
    wt_in = nc.dram_tensor("wt", [K, N], F32, kind="ExternalInput")
    xt_in = nc.dram_tensor("xt", [K, BC], F32R, kind="ExternalInput")
    bias_in = nc.dram_tensor("bias", [128, N], F32, kind="ExternalInput")
    mmx_in = nc.dram_tensor("mmx", [128, 20], F32, kind="ExternalInput")
    cv_in = nc.dram_tensor("cv", [128, 2 * N], F32, kind="ExternalInput")
    out_d = nc.dram_tensor("out", [BC, N], F32, kind="ExternalOutput")

    # x group g holds k rows [g*512, (g+1)*512) as [128p, 4kt, 1024m]
    xt_r = xt_in.rearrange("(g j p) m -> g p j m", g=XG, p=128)
    # wt viewed as [128p, kt, n] so one DMA fills a [128, 2, W] kt-pair tile
    wt_r = wt_in.rearrange("(kt p) n -> p kt n", p=128)
    cv_r = cv_in.rearrange("p (two n) -> p two n", two=2)

    with tile.TileContext(nc) as tc:
        with (
            tc.tile_pool(name="xkp", bufs=1) as xkp,
            tc.tile_pool(name="cvp", bufs=1) as cvp,
            tc.tile_pool(name="biasp", bufs=1) as biasp,
            tc.tile_pool(name="smallp", bufs=1) as sp,
            tc.tile_pool(name="wtp", bufs=12) as wtp,
            tc.tile_pool(name="geffp", bufs=2) as geffp,
            tc.tile_pool(name="osbp", bufs=2) as osbp,
            tc.tile_pool(name="pcp", bufs=1, space="PSUM") as pcp,
        ):
            # ---------------- tiny inputs (host pre-broadcast) ----------------
            with nc.named_scope("setup"):
                bcv = sp.tile([128, 20], F32, tag="bcv")
                nc.sync.dma_start(out=bcv[:], in_=mmx_in[:])
                cv2s = cvp.tile([128, 2, N], F32, tag="cv2s")
                nc.gpsimd.dma_start(out=cv2s[:], in_=cv_r[:])
                bias_sb = biasp.tile([128, N], F32, tag="bias_sb")
                # warm the engines during the preamble: first custom-DVE op
                # triggers a library load (~5us) and first activation a table
                # load (~1.3us); pay both on a dummy tile before wt arrives.
                wu = sp.tile([128, 8], F32, tag="wu")
                nc.vector.memset(wu[:], 1.0)
                nc.vector._custom_dve(RECIPROCAL_APPROX_FAST, out=wu[:],
                                      in0=wu[:], s0=CRC["s0"], s1=CRC["s1"],
                                      imm2=CRC["imm2"])
                nc.scalar.activation(wu[:], wu[:], AF.Identity, bias=0.0,
                                     scale=1.0)
                nc.gpsimd.tensor_tensor(wu[:], wu[:], wu[:], ALU.add)
            kap_b = bcv[:, 2:3]
            rowb = bcv[:, 4:20]          # [128, kt]: s*(4098 - 256*kt - 2p)

            xg = {}
            geff = {}

            wtiles = {}

            def tf_front(ch, kps, interleave_x=False):
                """dma + recip1 + row-add for the given kt-PAIRS of chunk ch.

                Pair tiles [128, 2, W] halve the per-op overhead on the
                recip/col/kappa stages; the row-add bias differs per kt so it
                runs on per-kt slices."""
                for kp in kps:
                    w_t = wtp.tile([128, 2, W], F32, tag="wt",
                                   name=f"wt{ch}_{kp}")
                    nc.sync.dma_start(
                        out=w_t[:],
                        in_=wt_r[:, 2 * kp:2 * kp + 2,
                                 ch * W:(ch + 1) * W])
                    if interleave_x and kp % 2 == 1:
                        g = kp // 2
                        xg[g] = xkp.tile([128, 4, BC], F32R, tag=f"xg{g}",
                                         name=f"xg{g}")
                        nc.sync.dma_start(out=xg[g][:], in_=xt_r[g])
                    wtiles[ch, kp] = w_t
                for kp in kps:
                    nc.vector._custom_dve(RECIPROCAL_APPROX_FAST,
                                          out=wtiles[ch, kp][:],
                                          in0=wtiles[ch, kp][:],
                                          s0=CRC["s0"], s1=CRC["s1"],
                                          imm2=CRC["imm2"])
                for kp in kps:
                    for j in range(2):
                        kt = 2 * kp + j
                        nc.scalar.activation(wtiles[ch, kp][:, j, :],
                                             wtiles[ch, kp][:, j, :],
                                             AF.Identity,
                                             bias=rowb[:, kt:kt + 1],
                                             scale=1.0)

            def tf_back(ch):
                """col-add + recip2 + kappa for all kt-pairs of chunk ch."""
                KP = KT // 2
                for kp in range(KP):
                    nc.gpsimd.tensor_tensor(wtiles[ch, kp][:],
                                            wtiles[ch, kp][:],
                                            cv2s[:, :, ch * W:(ch + 1) * W],
                                            ALU.add)
                for kp in range(KP):
                    nc.vector._custom_dve(RECIPROCAL_APPROX_FAST,
                                          out=wtiles[ch, kp][:],
                                          in0=wtiles[ch, kp][:],
                                          s0=CRC["s0"], s1=CRC["s1"],
                                          imm2=CRC["imm2"])
                for kp in range(KP):
                    ge_t = geffp.tile([128, 2, W], F32R, tag=f"ge{kp}",
                                      name=f"ge{ch}_{kp}")
                    nc.scalar.activation(ge_t[:], wtiles[ch, kp][:],
                                         AF.Identity, bias=kap_b, scale=1.0)
                    geff[ch, kp] = ge_t

            def emit_epilogue(ch, mb, pc):
                # GPSIMD cannot read PSUM, so the bias add rides on DVE
                osb = osbp.tile([128, W], F32, tag="osb", name=f"ep{ch}_{mb}")
                nc.vector.tensor_tensor(osb[:], pc[:],
                                        bias_sb[:, ch * W:(ch + 1) * W],
                                        ALU.add)
                nc.sync.dma_start(
                    out=out_d[mb * 128:(mb + 1) * 128,
                              ch * W:(ch + 1) * W],
                    in_=osb[:])

            def one_mm(pc, ch, kt, mb):
                g, j = divmod(kt, 4)
                kp, h = divmod(kt, 2)
                nc.tensor.matmul(
                    pc[:], xg[g][:, j, mb * 128:(mb + 1) * 128],
                    geff[ch, kp][:, h, :], start=(kt == 0),
                    stop=(kt == KT - 1))

            def emit_mm(ch, kt_outer):
                pcs = {mb: pcp.tile([128, W], F32, tag=f"pc{mb}",
                                    name=f"pc{ch}_{mb}") for mb in range(MB)}
                if kt_outer:
                    # chunk 0: x and geff stream in k-order; consume them as
                    # they land so the PE never waits on the chunk tail.
                    for kt in range(KT):
                        for mb in range(MB):
                            one_mm(pcs[mb], ch, kt, mb)
                    for mb in range(MB):
                        emit_epilogue(ch, mb, pcs[mb])
                else:
                    # steady state: mb-outer staggers PSUM-bank drains so the
                    # epilogue and out-DMA spread across the chunk.
                    for mb in range(MB):
                        for kt in range(KT):
                            one_mm(pcs[mb], ch, kt, mb)
                        emit_epilogue(ch, mb, pcs[mb])

            KP = KT // 2
            HALF = KP // 2
            with nc.named_scope("tf0"):
                tf_front(0, range(KP), interleave_x=True)
            with nc.named_scope("tf1a"):
                tf_front(1, range(HALF))
            with nc.named_scope("tf0b"):
                tf_back(0)
            # bias lands well after the chunk-0 DMA crunch, before epilogue 0
            nc.sync.dma_start(out=bias_sb[:], in_=bias_in[:])
            # transform pipeline runs one full chunk ahead of the mm stream so
            # mm(ch)'s PSUM-draining epilogues never block tf(ch+1) recips
            with nc.named_scope("tf1b"):
                tf_front(1, range(HALF, KP))
                tf_front(2, range(HALF))
                tf_back(1)
            for ch in range(NCH):
                with nc.named_scope(f"mm{ch}"):
                    emit_mm(ch, kt_outer=(ch == 0))
                if ch + 2 < NCH:
                    with nc.named_scope(f"tf{ch + 2}b"):
                        tf_front(ch + 2, range(HALF, KP))
                        if ch + 3 < NCH:
                            tf_front(ch + 3, range(HALF))
                        tf_back(ch + 2)
    nc.finalize()
    return nc


def _prep_inputs(x, weight, bias):
    wmin = float(weight.min())
    wmax = float(weight.max())
    s = (G_MAX - G_MIN) / (wmax - wmin)
    a = G_MIN / s - wmin
    kappa = wmin - G_MIN / s
    # the +a shift rides on the wt relayout (transposed, contiguous, shifted)
    wtT = np.ascontiguousarray(weight.T + np.float32(a))
    mmx = np.zeros((128, 20), dtype=np.float32)
    mmx[:, 0] = s
    mmx[:, 1] = a
    mmx[:, 2] = kappa
    # per-(partition, ktile) parasitic term: s*(4098 - 256*kt - 2p)
    p = np.arange(128, dtype=np.float64)[:, None]
    kt = np.arange(KT, dtype=np.float64)[None, :]
    mmx[:, 4:20] = (s * (4098.0 - 256.0 * kt - 2.0 * p)).astype(np.float32)
    mmx = np.ascontiguousarray(mmx)
    # pure column part of the parasitic term: cv[p, n] = 2*s*n,
    # duplicated along a pair axis so it matches the [128, 2, W] kt-pair tiles
    cv1 = (np.float64(s) * 2.0 * np.arange(N, dtype=np.float64)).astype(np.float32)
    cv = np.ascontiguousarray(np.broadcast_to(
        np.tile(cv1, 2)[None, :], (128, 2 * N)))

    biasb = np.ascontiguousarray(
        np.broadcast_to(bias.reshape(1, N), (128, N))).astype(np.float32)
    in_maps = []
    for c in range(N_CORES):
        x_c = x[c * BC:(c + 1) * BC, :]           # [BC, K]
        xt_c = np.ascontiguousarray(x_c.T)
        in_maps.append({"wt": wtT, "xt": xt_c, "bias": biasb, "mmx": mmx,
                        "cv": cv})
    return in_maps


def _run(x, weight, bias, trace=False, trace_kwargs=None):
    if "nc" not in _CACHE:
        _CACHE["nc"] = _build_nc()
    nc = _CACHE["nc"]
    in_maps = _prep_inputs(x, weight, bias)
    res = run_bass_kernel_spmd(nc, in_maps, list(range(N_CORES)), trace=trace,
                               **(trace_kwargs or {}))
    out = np.concatenate([res.results[c]["out"] for c in range(N_CORES)], axis=0)
    return out, res


def kernel(x, weight, bias):
    x = np.asarray(x, dtype=np.float32)
    weight = np.asarray(weight, dtype=np.float32)
    bias = np.asarray(bias, dtype=np.float32)
    out, _ = _run(x, weight, bias, trace=False)
    return out.astype(np.float32)


# revision 17
# speedup vs baseline: 1.0594x; 1.0227x over previous
"""Trainium2 Bass kernel for the memristor-crossbar layer (nn_CustomLayer_30588757082254).

out = unmap(x @ G_eff) + bias, where G_eff = 1/(1/G + R_par) is an elementwise
transform of weight.T with globally min/max-normalized conductances.

Strategy: data-parallel over batch (8 cores x 1024 rows). Single-phase,
column-chunked pipeline: the N=2048 output columns are processed in 4 chunks
of 512; chunk ch+1's conductance transform runs on ACT/DVE/Pool while chunk
ch's matmuls run on the PE with full-K accumulation in PSUM (all 8 banks, one
per m-block), so there is no DRAM staging of partial products and x is loaded
exactly once.

Math (S = 1/s folds the unmapping scale; kappa = wmin - g_min*S = -a):
  s = (g_max-g_min)/(wmax-wmin);  a = g_min/s - wmin
  geff'  = 1/( 1/(WT+a) + s*R )        (R = 4098 + 2n - 2p - 256*kt)
  geff'' = geff' + kappa               (folds the xs*kappa row-sum term:
                                        x @ (kappa*ones) = xs*kappa)
  out    = x @ geff'' + bias

Folding kappa into the operand eliminates both the row-sum (xs) matmuls and
the rank-1 bias matmuls from the PE; bias is added in a DVE tensor-tensor
epilogue straight out of PSUM. The +a shift rides on the host's wt relayout
(wt input = weight.T + a). s*R is split into a per-(partition,ktile) part
(ACT bias table, [128,16]) and a pure column part 2*s*n (Pool tensor-tensor,
the only elementwise op GPSIMD supports - no PSUM, no TensorScalarPtr).
Transform per k-tile, in-place on the streamed wt tile:
  DVE recip -> ACT +row -> Pool +col -> DVE recip -> ACT +kappa (f32r out).
Each engine carries <= ~26us per 27.3us matmul chunk, so the PE stays the
bottleneck. Emission is stage-major per chunk (all recips, then all row-adds,
...) so the in-order engine queues pipeline across k-tiles; the wt ring must
therefore hold a full chunk (18 bufs > 16 k-tiles) or the queues deadlock.
"""
import numpy as np

import concourse.bass as bass
import concourse.mybir as mybir
import concourse.tile as tile
from concourse import bacc
from concourse.bass_utils import run_bass_kernel_spmd
from concourse.dve_ops import RECIP_APPROX_FAST_CONSTS, RECIPROCAL_APPROX_FAST

F32 = mybir.dt.float32
F32R = mybir.dt.float32r
AF = mybir.ActivationFunctionType
ALU = mybir.AluOpType
CRC = RECIP_APPROX_FAST_CONSTS

N_CORES = 8
B, K, N = 8192, 2048, 2048
BC = B // N_CORES            # 1024 batch rows per core
KT = K // 128                # 16 k-tiles
MB = BC // 128               # 8 m-blocks per core
# variable chunk widths: narrow chunk 1 trims the DMA-bound fill (less wt
# needed before mm1); the ramp keeps later transforms ahead of their mm
# deadline. All widths <= 512 (PSUM bank cap).
CW = [512, 256, 384, 448, 448]
CO = [sum(CW[:i]) for i in range(len(CW))]   # column offsets
NCH = len(CW)
XG = 4                       # x loaded in 4 groups of 4 k-tiles

PARASITIC_R = 2.0
G_MIN, G_MAX = 1.0 / 100000.0, 1.0 / 1000.0

_CACHE = {}


def _build_nc():
    nc = bacc.Bacc("TRN2", target_bir_lowering=False, debug=False,
                   num_devices=N_CORES)
    wt_in = nc.dram_tensor("wt", [K, N], F32, kind="ExternalInput")
    xt_in = nc.dram_tensor("xt", [K, BC], F32R, kind="ExternalInput")
    bias_in = nc.dram_tensor("bias", [128, N], F32, kind="ExternalInput")
    mmx_in = nc.dram_tensor("mmx", [128, 20], F32, kind="ExternalInput")
    cv_in = nc.dram_tensor("cv", [128, N], F32, kind="ExternalInput")
    out_d = nc.dram_tensor("out", [BC, N], F32, kind="ExternalOutput")

    # x group g holds k rows [g*512, (g+1)*512) as [128p, 4kt, 1024m]
    xt_r = xt_in.rearrange("(g j p) m -> g p j m", g=XG, p=128)

    with tile.TileContext(nc) as tc:
        with (
            tc.tile_pool(name="xkp", bufs=1) as xkp,
            tc.tile_pool(name="cvp", bufs=1) as cvp,
            tc.tile_pool(name="biasp", bufs=1) as biasp,
            tc.tile_pool(name="smallp", bufs=1) as sp,
            tc.tile_pool(name="wtp", bufs=26) as wtp,
            tc.tile_pool(name="geffp", bufs=2) as geffp,
            tc.tile_pool(name="osbp", bufs=4) as osbp,
            tc.tile_pool(name="pcp", bufs=1, space="PSUM") as pcp,
        ):
            # ---------------- tiny inputs (host pre-broadcast) ----------------
            with nc.named_scope("setup"):
                bcv = sp.tile([128, 20], F32, tag="bcv")
                nc.sync.dma_start(out=bcv[:], in_=mmx_in[:])
                cv2s = cvp.tile([128, N], F32, tag="cv2s")
                nc.gpsimd.dma_start(out=cv2s[:], in_=cv_in[:])
                bias_sb = biasp.tile([128, N], F32, tag="bias_sb")
                # warm the engines during the preamble: first custom-DVE op
                # triggers a library load (~5us) and first activation a table
                # load (~1.3us); pay both on a dummy tile before wt arrives.
                wu = sp.tile([128, 8], F32, tag="wu")
                nc.vector.memset(wu[:], 1.0)
                nc.vector._custom_dve(RECIPROCAL_APPROX_FAST, out=wu[:],
                                      in0=wu[:], s0=CRC["s0"], s1=CRC["s1"],
                                      imm2=CRC["imm2"])
                nc.scalar.activation(wu[:], wu[:], AF.Identity, bias=0.0,
                                     scale=1.0)
                nc.gpsimd.tensor_tensor(wu[:], wu[:], wu[:], ALU.add)
            kap_b = bcv[:, 2:3]
            rowb = bcv[:, 4:20]          # [128, kt]: s*(4098 - 256*kt - 2p)

            xg = {}
            geff = {}

            wtiles = {}

            def tf_front(ch, kts, interleave_x=False):
                """dma + recip1 + row-add for the given k-tiles of chunk ch."""
                for kt in kts:
                    w_t = wtp.tile([128, CW[ch]], F32, tag="wt",
                                   name=f"wt{ch}_{kt}")
                    nc.sync.dma_start(
                        out=w_t[:],
                        in_=wt_in[kt * 128:(kt + 1) * 128,
                                  CO[ch]:CO[ch] + CW[ch]])
                    if interleave_x and kt % 4 == 3:
                        g = kt // 4
                        xg[g] = xkp.tile([128, 4, BC], F32R, tag=f"xg{g}",
                                         name=f"xg{g}")
                        nc.sync.dma_start(out=xg[g][:], in_=xt_r[g])
                    wtiles[ch, kt] = w_t
                for kt in kts:
                    nc.vector._custom_dve(RECIPROCAL_APPROX_FAST,
                                          out=wtiles[ch, kt][:],
                                          in0=wtiles[ch, kt][:],
                                          s0=CRC["s0"], s1=CRC["s1"],
                                          imm2=CRC["imm2"])
                for kt in kts:
                    nc.scalar.activation(wtiles[ch, kt][:], wtiles[ch, kt][:],
                                         AF.Identity,
                                         bias=rowb[:, kt:kt + 1], scale=1.0)

            def tf_back(ch):
                """col-add + recip2 + kappa for all k-tiles of chunk ch."""
                for kt in range(KT):
                    nc.gpsimd.tensor_tensor(wtiles[ch, kt][:],
                                            wtiles[ch, kt][:],
                                            cv2s[:, CO[ch]:CO[ch] + CW[ch]],
                                            ALU.add)
                for kt in range(KT):
                    nc.vector._custom_dve(RECIPROCAL_APPROX_FAST,
                                          out=wtiles[ch, kt][:],
                                          in0=wtiles[ch, kt][:],
                                          s0=CRC["s0"], s1=CRC["s1"],
                                          imm2=CRC["imm2"])
                for kt in range(KT):
                    ge_t = geffp.tile([128, CW[ch]], F32R,
                                      tag=f"ge{kt}",
                                      name=f"ge{ch}_{kt}")
                    nc.scalar.activation(ge_t[:], wtiles[ch, kt][:],
                                         AF.Identity, bias=kap_b, scale=1.0)
                    geff[ch, kt] = ge_t

            def emit_epilogue(ch, mb, pc):
                # GPSIMD cannot read PSUM, so the bias add rides on DVE
                osb = osbp.tile([128, CW[ch]], F32, tag="osb",
                                name=f"ep{ch}_{mb}")
                nc.vector.tensor_tensor(osb[:], pc[:],
                                        bias_sb[:, CO[ch]:CO[ch] + CW[ch]],
                                        ALU.add)
                nc.sync.dma_start(
                    out=out_d[mb * 128:(mb + 1) * 128,
                              CO[ch]:CO[ch] + CW[ch]],
                    in_=osb[:])

            def one_mm(pc, ch, kt, mb):
                g, j = divmod(kt, 4)
                nc.tensor.matmul(
                    pc[:], xg[g][:, j, mb * 128:(mb + 1) * 128],
                    geff[ch, kt][:], start=(kt == 0), stop=(kt == KT - 1))

            def emit_mm(ch, kt_outer):
                pcs = {mb: pcp.tile([128, CW[ch]], F32, tag=f"pc{mb}",
                                    name=f"pc{ch}_{mb}") for mb in range(MB)}
                if kt_outer:
                    # chunk 0: x and geff stream in k-order; consume them as
                    # they land so the PE never waits on the chunk tail.
                    for kt in range(KT):
                        for mb in range(MB):
                            one_mm(pcs[mb], ch, kt, mb)
                    for mb in range(MB):
                        emit_epilogue(ch, mb, pcs[mb])
                else:
                    # steady state: mb-outer staggers PSUM-bank drains so the
                    # epilogue and out-DMA spread across the chunk.
                    for mb in range(MB):
                        for kt in range(KT):
                            one_mm(pcs[mb], ch, kt, mb)
                        emit_epilogue(ch, mb, pcs[mb])

            HALF = KT // 2
            with nc.named_scope("tf0"):
                tf_front(0, range(KT), interleave_x=True)
            with nc.named_scope("tf1a"):
                tf_front(1, range(HALF))
            with nc.named_scope("tf0b"):
                tf_back(0)
            # bias lands well after the chunk-0 DMA crunch, before epilogue 0
            nc.sync.dma_start(out=bias_sb[:], in_=bias_in[:])
            # transform pipeline runs one full chunk ahead of the mm stream so
            # mm(ch)'s PSUM-draining epilogues never block tf(ch+1) recips
            with nc.named_scope("tf1b"):
                tf_front(1, range(HALF, KT))
                tf_front(2, range(HALF))
                tf_back(1)
            for ch in range(NCH):
                with nc.named_scope(f"mm{ch}"):
                    emit_mm(ch, kt_outer=(ch == 0))
                if ch + 2 < NCH:
                    with nc.named_scope(f"tf{ch + 2}b"):
                        tf_front(ch + 2, range(HALF, KT))
                        if ch + 3 < NCH:
                            tf_front(ch + 3, range(HALF))
                        tf_back(ch + 2)
    nc.finalize()
    return nc


def _prep_inputs(x, weight, bias):
    wmin = float(weight.min())
    wmax = float(weight.max())
    s = (G_MAX - G_MIN) / (wmax - wmin)
    a = G_MIN / s - wmin
    kappa = wmin - G_MIN / s
    # the +a shift rides on the wt relayout (transposed, contiguous, shifted)
    wtT = np.ascontiguousarray(weight.T + np.float32(a))
    mmx = np.zeros((128, 20), dtype=np.float32)
    mmx[:, 0] = s
    mmx[:, 1] = a
    mmx[:, 2] = kappa
    # per-(partition, ktile) parasitic term: s*(4098 - 256*kt - 2p)
    p = np.arange(128, dtype=np.float64)[:, None]
    kt = np.arange(KT, dtype=np.float64)[None, :]
    mmx[:, 4:20] = (s * (4098.0 - 256.0 * kt - 2.0 * p)).astype(np.float32)
    mmx = np.ascontiguousarray(mmx)
    # pure column part of the parasitic term: cv[p, n] = 2*s*n
    cv = np.ascontiguousarray(np.broadcast_to(
        (np.float64(s) * 2.0 * np.arange(N, dtype=np.float64))[None, :]
        .astype(np.float32), (128, N)))

    biasb = np.ascontiguousarray(
        np.broadcast_to(bias.reshape(1, N), (128, N))).astype(np.float32)
    in_maps = []
    for c in range(N_CORES):
        x_c = x[c * BC:(c + 1) * BC, :]           # [BC, K]
        xt_c = np.ascontiguousarray(x_c.T)
        in_maps.append({"wt": wtT, "xt": xt_c, "bias": biasb, "mmx": mmx,
                        "cv": cv})
    return in_maps


def _run(x, weight, bias, trace=False, trace_kwargs=None):
    if "nc" not in _CACHE:
        _CACHE["nc"] = _build_nc()
    nc = _CACHE["nc"]
    in_maps = _prep_inputs(x, weight, bias)
    res = run_bass_kernel_spmd(nc, in_maps, list(range(N_CORES)), trace=trace,
                               **(trace_kwargs or {}))
    out = np.concatenate([res.results[c]["out"] for c in range(N_CORES)], axis=0)
    return out, res


def kernel(x, weight, bias):
    x = np.asarray(x, dtype=np.float32)
    weight = np.asarray(weight, dtype=np.float32)
    bias = np.asarray(bias, dtype=np.float32)
    out, _ = _run(x, weight, bias, trace=False)
    return out.astype(np.float32)
